# revision 12
# baseline (speedup 1.0000x reference)
"""Trainium2 Bass kernel for nn_MAEGIN (GIN message passing, 5 layers + decoder).

Strategy (8 NeuronCores, one chip):
- Nodes sharded contiguously: core c owns rows [c*6250, (c+1)*6250).
- Node features kept on-chip, feature-major (hT: [2][128 feat, 6272 nodes]) in
  fp32; matmul inputs in bf16.
- Each GIN aggregation: AllGather node features (bf16) into DRAM, then per
  128-dst chunk: hardware dma_gather of edge-source rows (edges on partitions),
  one-hot selection matrices built on DVE (is_equal vs iota), and PE matmuls
  accumulate segment sums in PSUM. Self term added via identity matmul.
- Edge metadata (per-chunk CSR, int16 gather indices in wrapped-16 layout,
  per-edge dst offsets) is precomputed on host; the schedule (tiles per chunk)
  is the max over cores so one SPMD program serves all 8 cores.
- BN(eval)+PReLU fused into one ScalarE activation (Prelu, per-partition
  scale/bias folded on host).
- Decoder: proj matmul, second aggregation, trn matmul (+bias via ACT),
  then [nodes x 4096] logits matmul with vocab bias added on DVE.
"""

import math
import numpy as np
import ml_dtypes

import concourse.bass as bass
import concourse.bacc as bacc
import concourse.tile as tile
import concourse.mybir as mybir
from concourse.bass_utils import run_bass_kernel_spmd

P = 128
D = 256
NCORES = 8
N_NODES = 50000
N_EDGES = 800000
VOCAB = 4096
L = 5
BN_EPS = 1e-5

NL = N_NODES // NCORES          # 6250 nodes per core
NCHUNK = math.ceil(NL / P)      # 49 dst chunks
NLP = NCHUNK * P                # 6272 padded nodes per core
LAST_VALID = NL - (NCHUNK - 1) * P  # 106 valid rows in last chunk
AGN = NLP * NCORES              # 50176 rows in allgathered buffer
SPLIT = 32768                   # int16 gather index split
V1OFF = AGN - 1 - 32767         # 17408: base row of second gather view
NT_SIZES = [512] * (NLP // 512) + ([NLP % 512] if NLP % 512 else [])  # 12x512+128

f32 = mybir.dt.float32
bf16 = mybir.dt.bfloat16
i16 = mybir.dt.int16

PAD_DOFF = 200.0  # dst offset for padding edges: matches no iota value -> zero row


def _wrap16(idx: np.ndarray) -> np.ndarray:
    """[L] int array -> [128, L//16] wrapped-16 int16, replicated across q7 groups."""
    Ln = len(idx)
    assert Ln % 16 == 0
    base = idx.reshape(Ln // 16, 16).T.astype(np.int16)
    return np.ascontiguousarray(np.tile(base, (8, 1)))


def _prepare(x, edge_index, emb, W1s, b1s, g1s, be1s, m1s, v1s, a1s,
             W2s, b2s, g2s, be2s, m2s, v2s, a2s, Wproj, Wtrn, btrn, Wprd, bprd):
    """Host-side sharding/preprocessing. Returns (schedule, in_maps)."""
    x = np.asarray(x).astype(np.int64)
    src = np.asarray(edge_index[0]).astype(np.int64)
    dst = np.asarray(edge_index[1]).astype(np.int64)

    # global AG row id for every edge source. ag_in is written partition-major
    # (h_nm[p, c, :] -> ag_in row p*NCHUNK+c), so local node j = c*128+p maps to
    # row (j % 128) * NCHUNK + j // 128 within its core block.
    loc = src % NL
    ag_row = (src // NL) * NLP + (loc % P) * NCHUNK + loc // P

    dst_core = dst // NL
    dst_loc = dst % NL
    chunk = dst_loc // P
    doff = dst_loc % P
    half = (ag_row >= SPLIT).astype(np.int64)

    # group edges per core by (chunk, half)
    counts = np.zeros((NCORES, NCHUNK, 2), dtype=np.int64)
    per_core = []
    for c in range(NCORES):
        m = dst_core == c
        key = chunk[m] * 2 + half[m]
        order = np.argsort(key, kind="stable")
        ksort = key[m.nonzero()[0][order]] if False else key[order]
        cnt = np.bincount(key, minlength=NCHUNK * 2).reshape(NCHUNK, 2)
        counts[c] = cnt
        per_core.append((ag_row[m][order], doff[m][order], cnt))

    tiles = np.maximum(np.ceil(counts.max(axis=0) / P).astype(np.int64), 0)  # [49,2]
    tiles_per_chunk = tiles.sum(axis=1)            # [49]
    TT = int(tiles_per_chunk.sum())                # total edge tiles
    TMAXC = int(tiles_per_chunk.max())
    EPAD = TT * P

    # per-(chunk,half) start offsets in padded edge space
    flat_tiles = tiles.reshape(-1)                 # [(p,h)] order
    tile_start = np.concatenate([[0], np.cumsum(flat_tiles)])[:-1].reshape(NCHUNK, 2)
    edge_start = tile_start * P

    eidx_maps, doff_maps = [], []
    for c in range(NCORES):
        rows_s, doff_s, cnt = per_core[c]
        grp_start = np.concatenate([[0], np.cumsum(cnt.reshape(-1))])
        idx_pad = np.zeros(EPAD, dtype=np.int64)
        doff_pad = np.full(EPAD, int(PAD_DOFF), dtype=np.int64)
        for p in range(NCHUNK):
            for h in range(2):
                g = p * 2 + h
                n = int(cnt[p, h])
                if n == 0:
                    continue
                s = int(edge_start[p, h])
                seg = rows_s[grp_start[g]:grp_start[g] + n]
                idx_pad[s:s + n] = seg - (V1OFF if h == 1 else 0)
                doff_pad[s:s + n] = doff_s[grp_start[g]:grp_start[g] + n]
        assert idx_pad.min() >= 0 and idx_pad.max() <= 32767
        eidx_maps.append(_wrap16(idx_pad))
        # [128, TT]: column t entry p = doff of edge t*128+p
        doff_maps.append(np.ascontiguousarray(
            doff_pad.reshape(TT, P).T.astype(ml_dtypes.bfloat16)))

    # embedding gather indices (per core, padded to NLP with 0)
    xw_maps = []
    for c in range(NCORES):
        xp = np.zeros(NLP, dtype=np.int64)
        xp[:NL] = x[c * NL:(c + 1) * NL]
        xw_maps.append(_wrap16(xp))

    # weights, shared across cores
    def lhst(W):  # W [out, in] -> [128, ki, mo, 128] with slice = W.T chunk
        o, i = W.shape
        ko, mo = i // P, o // P
        return np.ascontiguousarray(
            W.T.reshape(ko, P, mo, P).transpose(1, 0, 2, 3).astype(ml_dtypes.bfloat16))

    w1t = np.stack([lhst(W1s[l]) for l in range(L)])  # [L,128,2,2,128]
    w1t = np.ascontiguousarray(w1t.transpose(1, 0, 2, 3, 4))  # [128,L,2,2,128]
    w2t = np.stack([lhst(W2s[l]) for l in range(L)])
    w2t = np.ascontiguousarray(w2t.transpose(1, 0, 2, 3, 4))
    wpt = lhst(Wproj)
    wtt = lhst(Wtrn)
    wprdt = np.ascontiguousarray(
        Wprd.T.reshape(2, P, VOCAB).transpose(1, 0, 2).astype(ml_dtypes.bfloat16))

    # folded BN scale/shift: per (l, sub, half) -> col l*4+sub*2+half
    bnsc = np.zeros((P, L * 4), dtype=np.float32)
    bnsh = np.zeros((P, L * 4), dtype=np.float32)
    alph = np.zeros((P, L * 2), dtype=np.float32)
    for l in range(L):
        for sub, (g, be, m, v, a) in enumerate(
                [(g1s[l], be1s[l], m1s[l], v1s[l], a1s[l]),
                 (g2s[l], be2s[l], m2s[l], v2s[l], a2s[l])]):
            sc = (g / np.sqrt(v + BN_EPS)).astype(np.float32)
            sh = (be - m * sc).astype(np.float32)
            for mo in range(2):
                bnsc[:, l * 4 + sub * 2 + mo] = sc[mo * P:(mo + 1) * P]
                bnsh[:, l * 4 + sub * 2 + mo] = sh[mo * P:(mo + 1) * P]
            alph[:, l * 2 + sub] = float(np.asarray(a))
    btrn2 = np.ascontiguousarray(btrn.reshape(2, P).T.astype(np.float32))
    bprdb = np.ascontiguousarray(
        np.tile(bprd, (P, 1)).astype(ml_dtypes.bfloat16))

    iota = np.tile(np.arange(P, dtype=np.float32), (P, 1)).astype(ml_dtypes.bfloat16)
    identb = np.eye(P, dtype=np.float32).astype(ml_dtypes.bfloat16)
    identf = np.eye(P, dtype=np.float32)
    embt = np.asarray(emb).astype(ml_dtypes.bfloat16)

    # pack small constants into one array per dtype (avoids per-tile padding)
    # bf16 pack layout: iota[128] identb[128] bprd[4096] w1[2560] w2[2560]
    #                   wp[512] wt[512] wprd[8192] doff[TT]
    def flat(a):  # [128, ...] -> [128, X]
        return a.reshape(P, -1)
    bf_shared = [flat(iota), flat(identb), flat(bprdb), flat(w1t), flat(w2t),
                 flat(wpt), flat(wtt), flat(wprdt)]
    offs = {}
    o = 0
    for name, a in zip(["iota", "identb", "bprd", "w1", "w2", "wp", "wt",
                        "wprd"], bf_shared):
        offs[name] = o
        o += a.shape[1]
    offs["doff"] = o
    bfw = o + TT
    f32_shared = np.concatenate(
        [flat(identf), bnsc, bnsh, alph, btrn2], axis=1).astype(np.float32)
    offs["identf"], offs["bnsc"], offs["bnsh"], offs["alph"], offs["btrn"] = (
        0, 128, 128 + L * 4, 128 + L * 8, 128 + L * 10)
    f32w = f32_shared.shape[1]
    i16w = EPAD // 16 + NLP // 16  # eidx then xw

    in_maps = []
    for c in range(NCORES):
        bfp = np.concatenate(bf_shared + [doff_maps[c]], axis=1)
        i16p = np.concatenate([eidx_maps[c], xw_maps[c]], axis=1)
        in_maps.append(dict(
            bfp=np.ascontiguousarray(bfp.astype(ml_dtypes.bfloat16)),
            f32p=np.ascontiguousarray(f32_shared),
            i16p=np.ascontiguousarray(i16p.astype(np.int16)),
            embt=embt))

    sched = dict(tiles=tiles, tile_start=tile_start, TT=TT, TMAXC=TMAXC,
                 offs=offs, bfw=bfw, f32w=f32w, i16w=i16w)
    return sched, in_maps


def _build(sched):
    tiles = sched["tiles"]
    tile_start = sched["tile_start"]
    TT = sched["TT"]
    TMAXC = sched["TMAXC"]
    offs = sched["offs"]
    EPAD = TT * P

    nc = bacc.Bacc("TRN2", target_bir_lowering=False)

    bfp = nc.dram_tensor("bfp", [P, sched["bfw"]], bf16, kind="ExternalInput")
    f32p = nc.dram_tensor("f32p", [P, sched["f32w"]], f32, kind="ExternalInput")
    i16p = nc.dram_tensor("i16p", [P, sched["i16w"]], i16, kind="ExternalInput")
    embt = nc.dram_tensor("embt", [VOCAB, D], bf16, kind="ExternalInput")

    out = nc.dram_tensor("out", [NL, VOCAB], f32, kind="ExternalOutput")

    ag_in = nc.dram_tensor("ag_in", [NLP, D], bf16)
    ag_out = nc.dram_tensor("ag_out", [AGN, D], bf16, addr_space="Shared")

    with tile.TileContext(nc) as tc:
        with (
            tc.tile_pool(name="cst", bufs=1) as cst,
            tc.tile_pool(name="gp", bufs=2) as gp,
            tc.tile_pool(name="selp", bufs=2) as selp,
            tc.tile_pool(name="accsb", bufs=2) as accsb,
            tc.tile_pool(name="op", bufs=3) as op,
            tc.tile_pool(name="accp", bufs=2, space="PSUM") as accp,
            tc.tile_pool(name="trp", bufs=2, space="PSUM") as trp,
            tc.tile_pool(name="mmp", bufs=2, space="PSUM") as mmp,
        ):
            bf_sb = cst.tile([P, sched["bfw"]], bf16, tag="bf")
            nc.sync.dma_start(bf_sb[:], bfp[:])
            f_sb = cst.tile([P, sched["f32w"]], f32, tag="f32")
            nc.sync.dma_start(f_sb[:], f32p[:])
            i_sb = cst.tile([P, sched["i16w"]], i16, tag="i16")
            nc.sync.dma_start(i_sb[:], i16p[:])

            def bfs(name, j0, w):
                s = offs[name] + j0
                return bf_sb[:, s:s + w]

            iota_sb = bfs("iota", 0, P)
            identb_sb = bfs("identb", 0, P)
            identf_sb = f_sb[:, offs["identf"]:offs["identf"] + P]

            def w_ap(name, l, ki, mo):
                return bfs(name, ((l * 2 + ki) * 2 + mo) * P, P)

            def bn_ap(kind, i):
                s = offs[kind] + i
                return f_sb[:, s:s + 1]

            # big resident feature tiles
            h_nm = cst.tile([P, NCHUNK, D], bf16, tag="h_nm")   # node-major
            hT = [cst.tile([P, NLP], f32, tag=f"hT{f}", name=f"hT{f}")
                  for f in range(2)]
            aggT = [cst.tile([P, NLP], bf16, tag=f"aggT{f}", name=f"aggT{f}")
                    for f in range(2)]

            # ---- embedding gather -> h_nm (bf16), build hT (fp32) ----
            xw_ap = i_sb[:, EPAD // 16: EPAD // 16 + NLP // 16]
            nc.gpsimd.dma_gather(h_nm[:], embt[:], xw_ap, NLP, NLP, D,
                                 single_packet=False)
            for p in range(NCHUNK):
                for fh in range(2):
                    tp = trp.tile([P, P], bf16, space="PSUM", tag="trb")
                    nc.tensor.transpose(
                        tp[:], h_nm[:, p, fh * P:(fh + 1) * P], identb_sb)
                    nc.vector.tensor_copy(hT[fh][:, p * P:(p + 1) * P], tp[:])

            def do_agg(src_nm, dst_fm):
                """GIN aggregate: dst_fm[f][feat,node] = sum_{j->i} h[j] + h[i]."""
                for p in range(NCHUNK):
                    T0, T1 = int(tiles[p, 0]), int(tiles[p, 1])
                    Tp = T0 + T1
                    ts0 = int(tile_start[p, 0])
                    g = gp.tile([P, TMAXC, D], bf16, tag="g")
                    if T0:
                        c0 = ts0 * 8
                        nc.gpsimd.dma_gather(
                            g[:, :T0, :], ag_out[:], i_sb[:, c0:c0 + T0 * 8],
                            T0 * P, T0 * P, D, single_packet=(T0 * P <= 1024))
                    if T1:
                        c1 = int(tile_start[p, 1]) * 8
                        nc.gpsimd.dma_gather(
                            g[:, T0:Tp, :], ag_out[V1OFF:, :],
                            i_sb[:, c1:c1 + T1 * 8], T1 * P, T1 * P, D,
                            single_packet=(T1 * P <= 1024))
                    sel = selp.tile([P, TMAXC, P], bf16, tag="sel")
                    dslc = bf_sb[:, offs["doff"] + ts0: offs["doff"] + ts0 + Tp]
                    nc.vector.tensor_tensor(
                        out=sel[:, :Tp, :],
                        in0=dslc[:, :, None].to_broadcast([P, Tp, P]),
                        in1=iota_sb[:, None, :].to_broadcast([P, Tp, P]),
                        op=mybir.AluOpType.is_equal,
                    )
                    acc = accp.tile([P, D], f32, space="PSUM", tag="acc")
                    for t in range(Tp):
                        nc.tensor.matmul(
                            out=acc[:], lhsT=sel[:, t, :], rhs=g[:, t, :],
                            start=(t == 0), stop=False)
                    nc.tensor.matmul(
                        out=acc[:], lhsT=identb_sb, rhs=src_nm[:, p, :],
                        start=False, stop=True)
                    asb = accsb.tile([P, D], bf16, tag="asb")
                    nc.vector.tensor_copy(asb[:], acc[:])
                    for fh in range(2):
                        tp = trp.tile([P, P], bf16, space="PSUM", tag="trb")
                        nc.tensor.transpose(
                            tp[:], asb[:, fh * P:(fh + 1) * P], identb_sb)
                        nc.vector.tensor_copy(
                            dst_fm[fh][:, p * P:(p + 1) * P], tp[:])

            def nsl(n):
                s = sum(NT_SIZES[:n])
                return slice(s, s + NT_SIZES[n])

            # ---- 5 GIN layers ----
            for l in range(L):
                nc.sync.dma_start(
                    ag_in[:].rearrange("(p c) f -> p c f", p=P), h_nm[:])
                nc.gpsimd.collective_compute(
                    "AllGather", mybir.AluOpType.bypass,
                    replica_groups=[list(range(NCORES))],
                    ins=[ag_in[:]], outs=[ag_out[:]])
                do_agg(h_nm, aggT)

                for n in range(len(NT_SIZES)):
                    w = NT_SIZES[n]
                    o1s = op.tile([P, 2, 512], bf16, tag="o1s")
                    for mo in range(2):
                        pm = mmp.tile([P, 512], f32, space="PSUM", tag="pm")
                        nc.tensor.matmul(pm[:, :w], w_ap("w1", l, 0, mo),
                                         aggT[0][:, nsl(n)], start=True, stop=False)
                        nc.tensor.matmul(pm[:, :w], w_ap("w1", l, 1, mo),
                                         aggT[1][:, nsl(n)], start=False, stop=True)
                        i = l * 4 + 0 * 2 + mo
                        nc.scalar.activation(
                            out=o1s[:, mo, :w], in_=pm[:, :w],
                            func=mybir.ActivationFunctionType.Prelu,
                            bias=bn_ap("bnsh", i), scale=bn_ap("bnsc", i),
                            alpha=bn_ap("alph", l * 2))
                    for mo in range(2):
                        pm = mmp.tile([P, 512], f32, space="PSUM", tag="pm")
                        nc.tensor.matmul(pm[:, :w], w_ap("w2", l, 0, mo),
                                         o1s[:, 0, :w], start=True, stop=False)
                        nc.tensor.matmul(pm[:, :w], w_ap("w2", l, 1, mo),
                                         o1s[:, 1, :w], start=False, stop=True)
                        i = l * 4 + 1 * 2 + mo
                        o2 = op.tile([P, 512], f32, tag="o2")
                        nc.scalar.activation(
                            out=o2[:, :w], in_=pm[:, :w],
                            func=mybir.ActivationFunctionType.Prelu,
                            bias=bn_ap("bnsh", i), scale=bn_ap("bnsc", i),
                            alpha=bn_ap("alph", l * 2 + 1))
                        nc.vector.tensor_tensor(
                            out=hT[mo][:, nsl(n)], in0=hT[mo][:, nsl(n)],
                            in1=o2[:, :w], op=mybir.AluOpType.add)
                if l < L - 1:
                    for p in range(NCHUNK):
                        for fh in range(2):
                            tp = trp.tile([P, P], f32, space="PSUM", tag="trf")
                            nc.tensor.transpose(
                                tp[:], hT[fh][:, p * P:(p + 1) * P], identf_sb)
                            nc.vector.tensor_copy(
                                h_nm[:, p, fh * P:(fh + 1) * P], tp[:])

            # ---- decoder ----
            for fh in range(2):
                nc.vector.tensor_copy(aggT[fh][:], hT[fh][:])
            # hT is dead after the proj matmuls; decoder temps share its slots
            decT = [cst.tile([P, NLP], bf16, tag=f"hT{f}", name=f"decT{f}")
                    for f in range(2)]
            for mo in range(2):
                for n in range(len(NT_SIZES)):
                    w = NT_SIZES[n]
                    pm = mmp.tile([P, 512], f32, space="PSUM", tag="pm")
                    nc.tensor.matmul(pm[:, :w], w_ap("wp", 0, 0, mo),
                                     aggT[0][:, nsl(n)], start=True, stop=False)
                    nc.tensor.matmul(pm[:, :w], w_ap("wp", 0, 1, mo),
                                     aggT[1][:, nsl(n)], start=False, stop=True)
                    nc.scalar.activation(
                        out=decT[mo][:, nsl(n)], in_=pm[:, :w],
                        func=mybir.ActivationFunctionType.Copy)
            for p in range(NCHUNK):
                for fh in range(2):
                    tp = trp.tile([P, P], bf16, space="PSUM", tag="trb")
                    nc.tensor.transpose(
                        tp[:], decT[fh][:, p * P:(p + 1) * P], identb_sb)
                    nc.vector.tensor_copy(h_nm[:, p, fh * P:(fh + 1) * P], tp[:])
            nc.sync.dma_start(
                ag_in[:].rearrange("(p c) f -> p c f", p=P), h_nm[:])
            nc.gpsimd.collective_compute(
                "AllGather", mybir.AluOpType.bypass,
                replica_groups=[list(range(NCORES))],
                ins=[ag_in[:]], outs=[ag_out[:]])
            do_agg(h_nm, aggT)

            # trn: tT = aggT @ Wtrn.T + btrn -> write into o1T
            for mo in range(2):
                for n in range(len(NT_SIZES)):
                    w = NT_SIZES[n]
                    pm = mmp.tile([P, 512], f32, space="PSUM", tag="pm")
                    nc.tensor.matmul(pm[:, :w], w_ap("wt", 0, 0, mo),
                                     aggT[0][:, nsl(n)], start=True, stop=False)
                    nc.tensor.matmul(pm[:, :w], w_ap("wt", 0, 1, mo),
                                     aggT[1][:, nsl(n)], start=False, stop=True)
                    nc.scalar.activation(
                        out=decT[mo][:, nsl(n)], in_=pm[:, :w],
                        func=mybir.ActivationFunctionType.Identity,
                        bias=bn_ap("btrn", mo), scale=1.0)

            # prd: logits, node-major output
            NV = VOCAB // 512
            for p in range(NCHUNK):
                rows = P if p < NCHUNK - 1 else LAST_VALID
                for v in range(NV):
                    vs = slice(v * 512, (v + 1) * 512)
                    pm = mmp.tile([P, 512], f32, space="PSUM", tag="pm")
                    nc.tensor.matmul(pm[:], decT[0][:, p * P:(p + 1) * P],
                                     bfs("wprd", 0 * VOCAB + v * 512, 512),
                                     start=True, stop=False)
                    nc.tensor.matmul(pm[:], decT[1][:, p * P:(p + 1) * P],
                                     bfs("wprd", 1 * VOCAB + v * 512, 512),
                                     start=False, stop=True)
                    ob = op.tile([P, 512], f32, tag="ob")
                    nc.vector.tensor_tensor(
                        out=ob[:], in0=pm[:], in1=bfs("bprd", v * 512, 512),
                        op=mybir.AluOpType.add)
                    nc.sync.dma_start(out[p * P:p * P + rows, vs], ob[:rows, :])

    nc.compile()
    return nc


def run_sharded(inputs: dict, trace: bool = False, trace_kwargs=None, tmpdir=None):
    sched, in_maps = _prepare(**inputs)
    nc = _build(sched)
    kw = {}
    if trace:
        kw = dict(trace=True, tmpdir=tmpdir)
        if trace_kwargs:
            kw["trace_kwargs"] = trace_kwargs
    res = run_bass_kernel_spmd(nc, in_maps, core_ids=list(range(NCORES)), **kw)
    full = np.concatenate([res.results[c]["out"] for c in range(NCORES)], axis=0)
    return full, res


def kernel(**inputs) -> np.ndarray:
    out, _ = run_sharded(inputs, trace=False)
    return out


# revision 13
# speedup vs baseline: 1.5339x; 1.5339x over previous
"""Trainium2 Bass kernel for nn_MAEGIN (GIN message passing, 5 layers + decoder).

Strategy (8 NeuronCores, one chip):
- Nodes sharded contiguously: core c owns rows [c*6250, (c+1)*6250).
- Node features kept on-chip, feature-major (hT: [2][128 feat, 6272 nodes]) in
  fp32; matmul inputs in bf16.
- Each GIN aggregation: AllGather node features (bf16) into DRAM, then per
  128-dst chunk: hardware dma_gather of edge-source rows (edges on partitions),
  one-hot selection matrices built on DVE (is_equal vs iota), and PE matmuls
  accumulate segment sums in PSUM. Self term added via identity matmul.
- Edge metadata (per-chunk CSR, int16 gather indices in wrapped-16 layout,
  per-edge dst offsets) is precomputed on host; the schedule (tiles per chunk)
  is the max over cores so one SPMD program serves all 8 cores.
- BN(eval)+PReLU fused into one ScalarE activation (Prelu, per-partition
  scale/bias folded on host).
- Decoder: proj matmul, second aggregation, trn matmul (+bias via ACT),
  then [nodes x 4096] logits matmul with vocab bias added on DVE.
"""

import math
import numpy as np
import ml_dtypes

import concourse.bass as bass
import concourse.bacc as bacc
import concourse.tile as tile
import concourse.mybir as mybir
from concourse.bass_utils import run_bass_kernel_spmd

P = 128
D = 256
NCORES = 8
N_NODES = 50000
N_EDGES = 800000
VOCAB = 4096
L = 5
BN_EPS = 1e-5

NL = N_NODES // NCORES          # 6250 nodes per core
NCHUNK = math.ceil(NL / P)      # 49 dst chunks
NLP = NCHUNK * P                # 6272 padded nodes per core
LAST_VALID = NL - (NCHUNK - 1) * P  # 106 valid rows in last chunk
AGN = NLP * NCORES              # 50176 rows in allgathered buffer
SPLIT = 32768                   # int16 gather index split
V1OFF = AGN - 1 - 32767         # 17408: base row of second gather view
NT_SIZES = [512] * (NLP // 512) + ([NLP % 512] if NLP % 512 else [])  # 12x512+128

f32 = mybir.dt.float32
bf16 = mybir.dt.bfloat16
i16 = mybir.dt.int16

PAD_DOFF = 200.0  # dst offset for padding edges: matches no iota value -> zero row


def _wrap16(idx: np.ndarray) -> np.ndarray:
    """[L] int array -> [128, L//16] wrapped-16 int16, replicated across q7 groups."""
    Ln = len(idx)
    assert Ln % 16 == 0
    base = idx.reshape(Ln // 16, 16).T.astype(np.int16)
    return np.ascontiguousarray(np.tile(base, (8, 1)))


def _prepare(x, edge_index, emb, W1s, b1s, g1s, be1s, m1s, v1s, a1s,
             W2s, b2s, g2s, be2s, m2s, v2s, a2s, Wproj, Wtrn, btrn, Wprd, bprd):
    """Host-side sharding/preprocessing. Returns (schedule, in_maps)."""
    x = np.asarray(x).astype(np.int64)
    src = np.asarray(edge_index[0]).astype(np.int64)
    dst = np.asarray(edge_index[1]).astype(np.int64)

    # global AG row id for every edge source. ag_in is written partition-major
    # (h_nm[p, c, :] -> ag_in row p*NCHUNK+c), so local node j = c*128+p maps to
    # row (j % 128) * NCHUNK + j // 128 within its core block.
    loc = src % NL
    ag_row = (src // NL) * NLP + (loc % P) * NCHUNK + loc // P

    dst_core = dst // NL
    dst_loc = dst % NL
    chunk = dst_loc // P
    doff = dst_loc % P
    half = (ag_row >= SPLIT).astype(np.int64)

    # group edges per core by (chunk, half)
    counts = np.zeros((NCORES, NCHUNK, 2), dtype=np.int64)
    per_core = []
    for c in range(NCORES):
        m = dst_core == c
        key = chunk[m] * 2 + half[m]
        order = np.argsort(key, kind="stable")
        ksort = key[m.nonzero()[0][order]] if False else key[order]
        cnt = np.bincount(key, minlength=NCHUNK * 2).reshape(NCHUNK, 2)
        counts[c] = cnt
        per_core.append((ag_row[m][order], doff[m][order], cnt))

    tiles = np.maximum(np.ceil(counts.max(axis=0) / P).astype(np.int64), 0)  # [49,2]
    tiles_per_chunk = tiles.sum(axis=1)            # [49]
    TT = int(tiles_per_chunk.sum())                # total edge tiles
    TMAXC = int(tiles_per_chunk.max())
    EPAD = TT * P

    # per-(chunk,half) start offsets in padded edge space
    flat_tiles = tiles.reshape(-1)                 # [(p,h)] order
    tile_start = np.concatenate([[0], np.cumsum(flat_tiles)])[:-1].reshape(NCHUNK, 2)
    edge_start = tile_start * P

    eidx_maps, doff_maps = [], []
    for c in range(NCORES):
        rows_s, doff_s, cnt = per_core[c]
        grp_start = np.concatenate([[0], np.cumsum(cnt.reshape(-1))])
        idx_pad = np.zeros(EPAD, dtype=np.int64)
        doff_pad = np.full(EPAD, int(PAD_DOFF), dtype=np.int64)
        for p in range(NCHUNK):
            for h in range(2):
                g = p * 2 + h
                n = int(cnt[p, h])
                if n == 0:
                    continue
                s = int(edge_start[p, h])
                seg = rows_s[grp_start[g]:grp_start[g] + n]
                idx_pad[s:s + n] = seg - (V1OFF if h == 1 else 0)
                doff_pad[s:s + n] = doff_s[grp_start[g]:grp_start[g] + n]
        assert idx_pad.min() >= 0 and idx_pad.max() <= 32767
        eidx_maps.append(_wrap16(idx_pad))
        # [128, TT]: column t entry p = doff of edge t*128+p
        doff_maps.append(np.ascontiguousarray(
            doff_pad.reshape(TT, P).T.astype(ml_dtypes.bfloat16)))

    # embedding gather indices (per core, padded to NLP with 0)
    xw_maps = []
    for c in range(NCORES):
        xp = np.zeros(NLP, dtype=np.int64)
        xp[:NL] = x[c * NL:(c + 1) * NL]
        xw_maps.append(_wrap16(xp))

    # weights, shared across cores
    def lhst(W):  # W [out, in] -> [128, ki, mo, 128] with slice = W.T chunk
        o, i = W.shape
        ko, mo = i // P, o // P
        return np.ascontiguousarray(
            W.T.reshape(ko, P, mo, P).transpose(1, 0, 2, 3).astype(ml_dtypes.bfloat16))

    w1t = np.stack([lhst(W1s[l]) for l in range(L)])  # [L,128,2,2,128]
    w1t = np.ascontiguousarray(w1t.transpose(1, 0, 2, 3, 4))  # [128,L,2,2,128]
    w2t = np.stack([lhst(W2s[l]) for l in range(L)])
    w2t = np.ascontiguousarray(w2t.transpose(1, 0, 2, 3, 4))
    wpt = lhst(Wproj)
    wtt = lhst(Wtrn)
    wprdt = np.ascontiguousarray(
        Wprd.T.reshape(2, P, VOCAB).transpose(1, 0, 2).astype(ml_dtypes.bfloat16))

    # folded BN scale/shift: per (l, sub, half) -> col l*4+sub*2+half
    bnsc = np.zeros((P, L * 4), dtype=np.float32)
    bnsh = np.zeros((P, L * 4), dtype=np.float32)
    alph = np.zeros((P, L * 2), dtype=np.float32)
    for l in range(L):
        for sub, (g, be, m, v, a) in enumerate(
                [(g1s[l], be1s[l], m1s[l], v1s[l], a1s[l]),
                 (g2s[l], be2s[l], m2s[l], v2s[l], a2s[l])]):
            sc = (g / np.sqrt(v + BN_EPS)).astype(np.float32)
            sh = (be - m * sc).astype(np.float32)
            for mo in range(2):
                bnsc[:, l * 4 + sub * 2 + mo] = sc[mo * P:(mo + 1) * P]
                bnsh[:, l * 4 + sub * 2 + mo] = sh[mo * P:(mo + 1) * P]
            alph[:, l * 2 + sub] = float(np.asarray(a))
    btrn2 = np.ascontiguousarray(btrn.reshape(2, P).T.astype(np.float32))
    bprdb = np.ascontiguousarray(
        np.tile(bprd, (P, 1)).astype(ml_dtypes.bfloat16))

    iota = np.tile(np.arange(P, dtype=np.float32), (P, 1)).astype(ml_dtypes.bfloat16)
    identb = np.eye(P, dtype=np.float32).astype(ml_dtypes.bfloat16)
    identf = np.eye(P, dtype=np.float32)
    embt = np.asarray(emb).astype(ml_dtypes.bfloat16)

    # pack small constants into one array per dtype (avoids per-tile padding)
    # bf16 pack layout: iota[128] identb[128] bprd[4096] w1[2560] w2[2560]
    #                   wp[512] wt[512] wprd[8192] doff[TT]
    def flat(a):  # [128, ...] -> [128, X]
        return a.reshape(P, -1)
    bf_shared = [flat(iota), flat(identb), flat(bprdb), flat(w1t), flat(w2t),
                 flat(wpt), flat(wtt), flat(wprdt)]
    offs = {}
    o = 0
    for name, a in zip(["iota", "identb", "bprd", "w1", "w2", "wp", "wt",
                        "wprd"], bf_shared):
        offs[name] = o
        o += a.shape[1]
    offs["doff"] = o
    bfw = o + TT
    f32_shared = np.concatenate(
        [flat(identf), bnsc, bnsh, alph, btrn2], axis=1).astype(np.float32)
    offs["identf"], offs["bnsc"], offs["bnsh"], offs["alph"], offs["btrn"] = (
        0, 128, 128 + L * 4, 128 + L * 8, 128 + L * 10)
    f32w = f32_shared.shape[1]
    i16w = EPAD // 16 + NLP // 16  # eidx then xw

    in_maps = []
    for c in range(NCORES):
        bfp = np.concatenate(bf_shared + [doff_maps[c]], axis=1)
        i16p = np.concatenate([eidx_maps[c], xw_maps[c]], axis=1)
        in_maps.append(dict(
            bfp=np.ascontiguousarray(bfp.astype(ml_dtypes.bfloat16)),
            f32p=np.ascontiguousarray(f32_shared),
            i16p=np.ascontiguousarray(i16p.astype(np.int16)),
            embt=embt))

    sched = dict(tiles=tiles, tile_start=tile_start, TT=TT, TMAXC=TMAXC,
                 offs=offs, bfw=bfw, f32w=f32w, i16w=i16w)
    return sched, in_maps


def _build(sched):
    tiles = sched["tiles"]
    tile_start = sched["tile_start"]
    TT = sched["TT"]
    TMAXC = sched["TMAXC"]
    offs = sched["offs"]
    EPAD = TT * P

    nc = bacc.Bacc("TRN2", target_bir_lowering=False, num_swdge_queues=4)

    bfp = nc.dram_tensor("bfp", [P, sched["bfw"]], bf16, kind="ExternalInput")
    f32p = nc.dram_tensor("f32p", [P, sched["f32w"]], f32, kind="ExternalInput")
    i16p = nc.dram_tensor("i16p", [P, sched["i16w"]], i16, kind="ExternalInput")
    embt = nc.dram_tensor("embt", [VOCAB, D], bf16, kind="ExternalInput")

    out = nc.dram_tensor("out", [NL, VOCAB], f32, kind="ExternalOutput")

    ag_in = nc.dram_tensor("ag_in", [NLP, D], bf16)
    ag_out = nc.dram_tensor("ag_out", [AGN, D], bf16, addr_space="Shared")

    with tile.TileContext(nc) as tc:
        with (
            tc.tile_pool(name="cst", bufs=1) as cst,
            tc.tile_pool(name="gp", bufs=3) as gp,
            tc.tile_pool(name="selp", bufs=3) as selp,
            tc.tile_pool(name="accsb", bufs=3) as accsb,
            tc.tile_pool(name="op", bufs=3) as op,
            tc.tile_pool(name="accp", bufs=3, space="PSUM") as accp,
            tc.tile_pool(name="trp", bufs=3, space="PSUM") as trp,
            tc.tile_pool(name="mmp", bufs=2, space="PSUM") as mmp,
        ):
            bf_sb = cst.tile([P, sched["bfw"]], bf16, tag="bf")
            nc.sync.dma_start(bf_sb[:], bfp[:])
            f_sb = cst.tile([P, sched["f32w"]], f32, tag="f32")
            nc.sync.dma_start(f_sb[:], f32p[:])
            i_sb = cst.tile([P, sched["i16w"]], i16, tag="i16")
            nc.sync.dma_start(i_sb[:], i16p[:])

            def bfs(name, j0, w):
                s = offs[name] + j0
                return bf_sb[:, s:s + w]

            iota_sb = bfs("iota", 0, P)
            identb_sb = bfs("identb", 0, P)
            identf_sb = f_sb[:, offs["identf"]:offs["identf"] + P]

            def w_ap(name, l, ki, mo):
                return bfs(name, ((l * 2 + ki) * 2 + mo) * P, P)

            def bn_ap(kind, i):
                s = offs[kind] + i
                return f_sb[:, s:s + 1]

            # big resident feature tiles
            h_nm = cst.tile([P, NCHUNK, D], bf16, tag="h_nm")   # node-major
            hT = [cst.tile([P, NLP], bf16, tag=f"hT{f}", name=f"hT{f}")
                  for f in range(2)]
            aggT = [cst.tile([P, NLP], bf16, tag=f"aggT{f}", name=f"aggT{f}")
                    for f in range(2)]

            # ---- embedding gather -> h_nm (bf16), build hT (fp32) ----
            xw_ap = i_sb[:, EPAD // 16: EPAD // 16 + NLP // 16]
            nc.gpsimd.dma_gather(h_nm[:], embt[:], xw_ap, NLP, NLP, D,
                                 single_packet=False)
            for p in range(NCHUNK):
                for fh in range(2):
                    tp = trp.tile([P, P], bf16, space="PSUM", tag="trb")
                    nc.tensor.transpose(
                        tp[:], h_nm[:, p, fh * P:(fh + 1) * P], identb_sb)
                    nc.vector.tensor_copy(hT[fh][:, p * P:(p + 1) * P], tp[:])

            qctr = [0]

            def nextq():
                qctr[0] = (qctr[0] + 1) % 4
                return qctr[0]

            def do_agg(src_nm, dst_fm):
                """GIN aggregate: dst_fm[f][feat,node] = sum_{j->i} h[j] + h[i]."""
                for p in range(NCHUNK):
                    T0, T1 = int(tiles[p, 0]), int(tiles[p, 1])
                    Tp = T0 + T1
                    ts0 = int(tile_start[p, 0])
                    g = gp.tile([P, TMAXC, D], bf16, tag="g")
                    if T0:
                        c0 = ts0 * 8
                        nc.gpsimd.dma_gather(
                            g[:, :T0, :], ag_out[:], i_sb[:, c0:c0 + T0 * 8],
                            T0 * P, T0 * P, D, single_packet=(T0 * P <= 1024),
                            queue_num=nextq())
                    if T1:
                        c1 = int(tile_start[p, 1]) * 8
                        nc.gpsimd.dma_gather(
                            g[:, T0:Tp, :], ag_out[V1OFF:, :],
                            i_sb[:, c1:c1 + T1 * 8], T1 * P, T1 * P, D,
                            single_packet=(T1 * P <= 1024))
                    sel = selp.tile([P, TMAXC, P], bf16, tag="sel")
                    dslc = bf_sb[:, offs["doff"] + ts0: offs["doff"] + ts0 + Tp]
                    nc.vector.tensor_tensor(
                        out=sel[:, :Tp, :],
                        in0=dslc[:, :, None].to_broadcast([P, Tp, P]),
                        in1=iota_sb[:, None, :].to_broadcast([P, Tp, P]),
                        op=mybir.AluOpType.is_equal,
                    )
                    acc = accp.tile([P, D], f32, space="PSUM", tag="acc")
                    for t in range(Tp):
                        nc.tensor.matmul(
                            out=acc[:], lhsT=sel[:, t, :], rhs=g[:, t, :],
                            start=(t == 0), stop=False)
                    nc.tensor.matmul(
                        out=acc[:], lhsT=identb_sb, rhs=src_nm[:, p, :],
                        start=False, stop=True)
                    asb = accsb.tile([P, D], bf16, tag="asb")
                    nc.vector.tensor_copy(asb[:], acc[:])
                    for fh in range(2):
                        tp = trp.tile([P, P], bf16, space="PSUM", tag="trb")
                        nc.tensor.transpose(
                            tp[:], asb[:, fh * P:(fh + 1) * P], identb_sb)
                        nc.vector.tensor_copy(
                            dst_fm[fh][:, p * P:(p + 1) * P], tp[:])

            def nsl(n):
                s = sum(NT_SIZES[:n])
                return slice(s, s + NT_SIZES[n])

            # ---- 5 GIN layers ----
            for l in range(L):
                nc.sync.dma_start(
                    ag_in[:].rearrange("(p c) f -> p c f", p=P), h_nm[:])
                nc.gpsimd.collective_compute(
                    "AllGather", mybir.AluOpType.bypass,
                    replica_groups=[list(range(NCORES))],
                    ins=[ag_in[:]], outs=[ag_out[:]])
                do_agg(h_nm, aggT)

                for n in range(len(NT_SIZES)):
                    w = NT_SIZES[n]
                    o1s = op.tile([P, 2, 512], bf16, tag="o1s")
                    for mo in range(2):
                        pm = mmp.tile([P, 512], f32, space="PSUM", tag="pm")
                        nc.tensor.matmul(pm[:, :w], w_ap("w1", l, 0, mo),
                                         aggT[0][:, nsl(n)], start=True, stop=False)
                        nc.tensor.matmul(pm[:, :w], w_ap("w1", l, 1, mo),
                                         aggT[1][:, nsl(n)], start=False, stop=True)
                        i = l * 4 + 0 * 2 + mo
                        nc.scalar.activation(
                            out=o1s[:, mo, :w], in_=pm[:, :w],
                            func=mybir.ActivationFunctionType.Prelu,
                            bias=bn_ap("bnsh", i), scale=bn_ap("bnsc", i),
                            alpha=bn_ap("alph", l * 2))
                    for mo in range(2):
                        pm = mmp.tile([P, 512], f32, space="PSUM", tag="pm")
                        nc.tensor.matmul(pm[:, :w], w_ap("w2", l, 0, mo),
                                         o1s[:, 0, :w], start=True, stop=False)
                        nc.tensor.matmul(pm[:, :w], w_ap("w2", l, 1, mo),
                                         o1s[:, 1, :w], start=False, stop=True)
                        i = l * 4 + 1 * 2 + mo
                        o2 = op.tile([P, 512], bf16, tag="o2")
                        nc.scalar.activation(
                            out=o2[:, :w], in_=pm[:, :w],
                            func=mybir.ActivationFunctionType.Prelu,
                            bias=bn_ap("bnsh", i), scale=bn_ap("bnsc", i),
                            alpha=bn_ap("alph", l * 2 + 1))
                        nc.vector.tensor_tensor(
                            out=hT[mo][:, nsl(n)], in0=hT[mo][:, nsl(n)],
                            in1=o2[:, :w], op=mybir.AluOpType.add)
                if l < L - 1:
                    for p in range(NCHUNK):
                        for fh in range(2):
                            tp = trp.tile([P, P], bf16, space="PSUM", tag="trb")
                            nc.tensor.transpose(
                                tp[:], hT[fh][:, p * P:(p + 1) * P], identb_sb)
                            nc.vector.tensor_copy(
                                h_nm[:, p, fh * P:(fh + 1) * P], tp[:])

            # ---- decoder ----
            for fh in range(2):
                nc.vector.tensor_copy(aggT[fh][:], hT[fh][:])
            # hT is dead after the proj matmuls; decoder temps share its slots
            decT = [cst.tile([P, NLP], bf16, tag=f"hT{f}", name=f"decT{f}")
                    for f in range(2)]
            for mo in range(2):
                for n in range(len(NT_SIZES)):
                    w = NT_SIZES[n]
                    pm = mmp.tile([P, 512], f32, space="PSUM", tag="pm")
                    nc.tensor.matmul(pm[:, :w], w_ap("wp", 0, 0, mo),
                                     aggT[0][:, nsl(n)], start=True, stop=False)
                    nc.tensor.matmul(pm[:, :w], w_ap("wp", 0, 1, mo),
                                     aggT[1][:, nsl(n)], start=False, stop=True)
                    nc.scalar.activation(
                        out=decT[mo][:, nsl(n)], in_=pm[:, :w],
                        func=mybir.ActivationFunctionType.Copy)
            for p in range(NCHUNK):
                for fh in range(2):
                    tp = trp.tile([P, P], bf16, space="PSUM", tag="trb")
                    nc.tensor.transpose(
                        tp[:], decT[fh][:, p * P:(p + 1) * P], identb_sb)
                    nc.vector.tensor_copy(h_nm[:, p, fh * P:(fh + 1) * P], tp[:])
            nc.sync.dma_start(
                ag_in[:].rearrange("(p c) f -> p c f", p=P), h_nm[:])
            nc.gpsimd.collective_compute(
                "AllGather", mybir.AluOpType.bypass,
                replica_groups=[list(range(NCORES))],
                ins=[ag_in[:]], outs=[ag_out[:]])
            do_agg(h_nm, aggT)

            # trn: tT = aggT @ Wtrn.T + btrn -> write into o1T
            for mo in range(2):
                for n in range(len(NT_SIZES)):
                    w = NT_SIZES[n]
                    pm = mmp.tile([P, 512], f32, space="PSUM", tag="pm")
                    nc.tensor.matmul(pm[:, :w], w_ap("wt", 0, 0, mo),
                                     aggT[0][:, nsl(n)], start=True, stop=False)
                    nc.tensor.matmul(pm[:, :w], w_ap("wt", 0, 1, mo),
                                     aggT[1][:, nsl(n)], start=False, stop=True)
                    nc.scalar.activation(
                        out=decT[mo][:, nsl(n)], in_=pm[:, :w],
                        func=mybir.ActivationFunctionType.Identity,
                        bias=bn_ap("btrn", mo), scale=1.0)

            # prd: logits, node-major output
            NV = VOCAB // 512
            for p in range(NCHUNK):
                rows = P if p < NCHUNK - 1 else LAST_VALID
                for v in range(NV):
                    vs = slice(v * 512, (v + 1) * 512)
                    pm = mmp.tile([P, 512], f32, space="PSUM", tag="pm")
                    nc.tensor.matmul(pm[:], decT[0][:, p * P:(p + 1) * P],
                                     bfs("wprd", 0 * VOCAB + v * 512, 512),
                                     start=True, stop=False)
                    nc.tensor.matmul(pm[:], decT[1][:, p * P:(p + 1) * P],
                                     bfs("wprd", 1 * VOCAB + v * 512, 512),
                                     start=False, stop=True)
                    ob = op.tile([P, 512], f32, tag="ob")
                    nc.vector.tensor_tensor(
                        out=ob[:], in0=pm[:], in1=bfs("bprd", v * 512, 512),
                        op=mybir.AluOpType.add)
                    nc.sync.dma_start(out[p * P:p * P + rows, vs], ob[:rows, :])

    nc.compile()
    return nc


def run_sharded(inputs: dict, trace: bool = False, trace_kwargs=None, tmpdir=None):
    sched, in_maps = _prepare(**inputs)
    nc = _build(sched)
    kw = {}
    if trace:
        kw = dict(trace=True, tmpdir=tmpdir)
        if trace_kwargs:
            kw["trace_kwargs"] = trace_kwargs
    res = run_bass_kernel_spmd(nc, in_maps, core_ids=list(range(NCORES)), **kw)
    full = np.concatenate([res.results[c]["out"] for c in range(NCORES)], axis=0)
    return full, res


def kernel(**inputs) -> np.ndarray:
    out, _ = run_sharded(inputs, trace=False)
    return out
